# revision 1
# baseline (speedup 1.0000x reference)
"""Block-sparse attention on 8 Trainium2 NeuronCores (Bass/Tile SPMD kernel).

Sharding: batch*head_groups across the 8 cores. Core c handles batch c//4 and
heads [4*(c%4), 4*(c%4)+4). Projection weights are sliced per core host-side
(pre-transposed + bf16-cast); the [16,16] block mask specializes the compiled
program (only kept blocks are computed). Each core emits a partial output
(its 256-wide d-slice pushed through Wo); the host sums the 4 partials per
batch and adds the bias.

Layout strategy per core (all attention math in "transposed" orientation):
  - x^T [1024, 2048] bf16 resident in SBUF (8 partition tiles)
  - q^T, k^T computed as [256, 2048] (2 pair-tiles of 128 partitions: each
    pair-tile stacks 2 heads of 64 rows); v stored as [v_h|1] blocks of 65
    cols per head so one M=65 matmul yields out2^T rows AND the softmax
    denominator row
  - attention sweeps k-blocks (j) outer within half-row chunks of 8 query
    blocks: the k^T_j stationary is loaded once per (head, j) and reused
    across that chunk's kept query blocks, with consecutive kept blocks
    fused into single wide-N matmuls (split only at PSUM bank boundaries)
  - one exp per (j, chunk) covers both heads' scores ([128, <=2048] PSUM)
  - normalize via reciprocal of the denominator row + DMA partition
    broadcast + two DVE multiplies into the bf16 out^T pair tile
  - final: out_partial[s, :] accumulates outTbf[p].T @ woT[p] over 2 pairs
"""

import time
from contextlib import ExitStack

import ml_dtypes
import numpy as np

import concourse.bass as bass
import concourse.tile as tile
from concourse import bacc, mybir
from concourse.ap import AP as APClass
from concourse.bass_utils import run_bass_kernel_spmd

BF16 = mybir.dt.bfloat16
F32 = mybir.dt.float32
bf16 = ml_dtypes.bfloat16

B, S, D, H = 2, 2048, 1024, 16
DH = 64
BLK = 128
NB = 16
NCORES = 8
HPC = H // (NCORES // B)   # 4 heads per core
E = HPC * DH               # 256 projection columns per core
EV = HPC * (DH + 1)        # 260: v stored as [v_h | ones] per head
KD = D // 128              # 8 contraction chunks

ALL_PHASES = ("proj", "attn", "final")

_nc_cache: dict = {}
last_run_info: dict = {}


def _bcast_ap(sl, n):
    """[1, W] SBUF slice -> [1, n, W] AP replicating the row n times (for DMA)."""
    apl = [list(x) for x in sl.ap]
    assert len(apl) == 2 and apl[0][1] == 1, apl
    return APClass(sl.tensor, sl.offset, [apl[0], [0, n], apl[1]])


def _runs_of(lst):
    out = []
    for i in lst:
        if out and i == out[-1][-1] + 1:
            out[-1].append(i)
        else:
            out.append([i])
    return out


def _emit(tc, aps, kept, loop_n=0, phases=ALL_PHASES):
    if loop_n:
        # measurement mode: run the whole body loop_n times on-device so the
        # per-iteration kernel time can be extracted from wall-clock diffs
        with tc.For_i(0, loop_n, 1):
            _emit_body(tc, aps, kept, phases)
    else:
        _emit_body(tc, aps, kept, phases)


def _emit_body(tc, aps, kept, phases=ALL_PHASES):
    nc = tc.nc
    xT_ap, wqT_ap, wkT_ap, wvT_ap, woT_ap, outp_ap = aps
    Exp = mybir.ActivationFunctionType.Exp

    with ExitStack() as ctx:
        persist = ctx.enter_context(tc.tile_pool(name="persist", bufs=1))

        # ---- Phase 0: load inputs -------------------------------------------------
        xT = []
        for kd in range(KD):
            t = persist.tile([128, S], BF16, name=f"xT{kd}", tag=f"xT{kd}")
            nc.sync.dma_start(t[:], xT_ap[kd * 128:(kd + 1) * 128, :])
            xT.append(t)

        def load_w(src_ap, name):
            ts = []
            for kd in range(KD):
                t = persist.tile([128, E], BF16, name=f"{name}{kd}", tag=f"{name}{kd}")
                nc.sync.dma_start(t[:], src_ap[kd * 128:(kd + 1) * 128, :])
                ts.append(t)
            return ts

        wq = load_w(wqT_ap, "wq")
        wk = load_w(wkT_ap, "wk")
        wv = load_w(wvT_ap, "wv")
        wo = []
        for p in range(2):
            t = persist.tile([128, D], BF16, name=f"wo{p}", tag=f"wo{p}")
            nc.sync.dma_start(t[:], woT_ap[p * 128:(p + 1) * 128, :])
            wo.append(t)

        zeros_sb = persist.tile([128, 512], BF16, name="zeros_sb", tag="zeros_sb")
        nc.any.memset(zeros_sb[:], 0.0)
        qT = [persist.tile([128, S], BF16, name=f"qT{p}", tag=f"qT{p}") for p in range(2)]
        kT = [persist.tile([128, S], BF16, name=f"kT{p}", tag=f"kT{p}") for p in range(2)]
        # v tiles: per s-block, layout [v_h0 | 1 | v_h1 | 1 | v_h2 | 1 | v_h3 | 1]
        vv = [persist.tile([128, EV], BF16, name=f"v{m}", tag=f"v{m}") for m in range(S // 128)]
        outTbf = [persist.tile([128, S], BF16, name=f"oT{p}", tag=f"oT{p}") for p in range(2)]

        if "proj" not in phases:
            return
        # ---- Phase 1: projections -------------------------------------------------
        with tc.tile_pool(name="proj_ps", bufs=8, space="PSUM") as proj_ps:
            # q^T and k^T: stationary = weight chunk, moving = x^T s-chunks
            for dst, w in ((qT, wq), (kT, wk)):
                for p in range(2):
                    pss = [proj_ps.tile([128, 512], F32, name="projps", tag="proj") for _ in range(4)]
                    for kd in range(KD):
                        for sc in range(4):
                            nc.tensor.matmul(
                                pss[sc][:],
                                w[kd][:, p * 128:(p + 1) * 128],
                                xT[kd][:, sc * 512:(sc + 1) * 512],
                                start=(kd == 0),
                                stop=(kd == KD - 1),
                            )
                    for sc in range(4):
                        nc.vector.tensor_copy(dst[p][:, sc * 512:(sc + 1) * 512], pss[sc][:])
            # v natural: stationary = x^T s-tile chunk, moving = wv
            for m in range(S // 128):
                ps = proj_ps.tile([128, 512], F32, name="projv", tag="proj")
                for kd in range(KD):
                    nc.tensor.matmul(
                        ps[:, 0:E],
                        xT[kd][:, m * 128:(m + 1) * 128],
                        wv[kd][:],
                        start=(kd == 0),
                        stop=(kd == KD - 1),
                    )
                # ones columns at 64, 129, 194, 259; v data strided around them
                v3 = vv[m].rearrange("p (g c) -> p g c", g=HPC)
                nc.any.memset(v3[:, :, 64:65], 1.0)
                nc.vector.tensor_copy(
                    v3[:, :, 0:64],
                    ps[:, 0:E].rearrange("p (g c) -> p g c", g=HPC),
                )

        if "attn" not in phases:
            # keep outputs defined for the runner
            nc.any.memset(outTbf[0][:, 0:128], 0.0)
            return
        # ---- Phase 2: block-sparse attention -------------------------------------
        # kept[i] = kept k-blocks (j) for query block i
        col_kept = [[i for i in range(NB) if j in kept[i]] for j in range(NB)]
        first_j = {i: kept[i][0] for i in range(NB)}
        last_j = {i: kept[i][-1] for i in range(NB)}

        with ExitStack() as actx:
            scA_pool = actx.enter_context(tc.tile_pool(name="scA_ps", bufs=1, space="PSUM"))
            scB_pool = actx.enter_context(tc.tile_pool(name="scB_ps", bufs=1, space="PSUM"))
            avA_pool = actx.enter_context(tc.tile_pool(name="avA_ps", bufs=1, space="PSUM"))
            avB_pool = actx.enter_context(tc.tile_pool(name="avB_ps", bufs=1, space="PSUM"))
            attn_pool = actx.enter_context(tc.tile_pool(name="attn_sb", bufs=4))
            norm_pool = actx.enter_context(tc.tile_pool(name="norm_sb", bufs=6))
            FILL = 1024  # per-head score fill width (2 PSUM banks)

            for p in range(2):
                for ch in range(2):
                    irange = list(range(ch * 8, ch * 8 + 8))
                    avA = [avA_pool.tile([65, 512], F32, name=f"avA{g}", tag=f"avA{g}")
                           for g in range(2)]
                    avB = [avB_pool.tile([65, 512], F32, name=f"avB{g}", tag=f"avB{g}")
                           for g in range(2)]
                    # zero-prime each av bank: one start=True matmul covering the
                    # full bank, so all region accumulations can use start=False
                    for t in avA + avB:
                        nc.tensor.matmul(t[:], vv[0][:, 0:65], zeros_sb[:],
                                         start=True, stop=False, skip_group_check=True)

                    def av_region(tiles, i):
                        li = i - ch * 8
                        return tiles[li // 4][:, (li % 4) * 128:(li % 4) * 128 + 128]

                    def flush(fill_js, scA, scB, colw):
                        # exp both heads, then their av contributions
                        atA = attn_pool.tile([128, FILL], BF16, name="atA", tag="at")
                        atB = attn_pool.tile([128, FILL], BF16, name="atB", tag="at")
                        nc.scalar.activation(atA[:, 0:colw], scA[:, 0:colw], Exp)
                        nc.scalar.activation(atB[:, 0:colw], scB[:, 0:colw], Exp)
                        for j, ks, off in fill_js:
                            for a in range(2):
                                h = 2 * p + a
                                lhs = vv[j][:, 65 * h:65 * h + 65]
                                tiles = avA if a == 0 else avB
                                at = atA if a == 0 else atB
                                for idx, i in enumerate(ks):
                                    nc.tensor.matmul(
                                        av_region(tiles, i),
                                        lhs,
                                        at[:, off + idx * 128: off + (idx + 1) * 128],
                                        start=False,
                                        stop=(j == last_j[i]),
                                        skip_group_check=True,
                                    )

                    fill_js = []
                    scA = scB = None
                    colw = 0
                    for j in range(NB):
                        ks = [i for i in col_kept[j] if i in irange]
                        if not ks:
                            continue
                        n = len(ks)
                        if scA is None or colw + n * 128 > FILL:
                            if scA is not None:
                                flush(fill_js, scA, scB, colw)
                            scA = scA_pool.tile([128, FILL], F32, name="scA", tag="scA")
                            scB = scB_pool.tile([128, FILL], F32, name="scB", tag="scB")
                            fill_js = []
                            colw = 0
                        # head A scores into scA, head B into scB (separate banks:
                        # concurrently-executing row-group-packed matmuls must not
                        # share a PSUM bank)
                        for a in range(2):
                            rows = slice(0, 64) if a == 0 else slice(64, 128)
                            dst = scA if a == 0 else scB
                            for run in _runs_of(ks):
                                idx0 = ks.index(run[0])
                                col = colw + idx0 * 128
                                width = len(run) * 128
                                qcol = run[0] * 128
                                done = 0
                                while done < width:
                                    seg = min(width - done, 512 - ((col + done) % 512))
                                    nc.tensor.matmul(
                                        dst[:, col + done: col + done + seg],
                                        kT[p][rows, j * 128:(j + 1) * 128],
                                        qT[p][rows, qcol + done: qcol + done + seg],
                                    )
                                    done += seg
                        fill_js.append((j, ks, colw))
                        colw += n * 128
                    if scA is not None:
                        flush(fill_js, scA, scB, colw)

                    # ---- normalization for this chunk ----
                    for i in irange:
                        icols = slice(i * 128, (i + 1) * 128)
                        stA = norm_pool.tile([65, 128], F32, name="stA", tag="stA")
                        nc.vector.tensor_copy(stA[:], av_region(avA, i))
                        stB = norm_pool.tile([65, 128], F32, name="stB", tag="stB")
                        nc.vector.tensor_copy(stB[:], av_region(avB, i))
                        rcA = norm_pool.tile([65, 128], F32, name="rcA", tag="rcA")
                        nc.vector.reciprocal(rcA[64:65, :], stA[64:65, :])
                        rcB = norm_pool.tile([65, 128], F32, name="rcB", tag="rcB")
                        nc.vector.reciprocal(rcB[64:65, :], stB[64:65, :])
                        bc = norm_pool.tile([128, 128], F32, name="bc", tag="bc")
                        nc.sync.dma_start(bc[0:64, :], _bcast_ap(rcA[64:65, :], 64))
                        nc.sync.dma_start(bc[64:128, :], _bcast_ap(rcB[64:65, :], 64))
                        stB2 = norm_pool.tile([128, 128], F32, name="stB2", tag="stB2")
                        nc.sync.dma_start(stB2[64:128, :], stB[0:64, :])
                        nc.vector.tensor_mul(outTbf[p][0:64, icols], stA[0:64, :], bc[0:64, :])
                        nc.vector.tensor_mul(outTbf[p][64:128, icols], stB2[64:128, :], bc[64:128, :])

        if "final" not in phases:
            return
        # ---- Phase 3: output projection (partial over this core's d-slice) -------
        with ExitStack() as fctx:
            fin_ps = fctx.enter_context(tc.tile_pool(name="fin_ps", bufs=4, space="PSUM"))
            fin_sb = fctx.enter_context(tc.tile_pool(name="fin_sb", bufs=4))
            for m in range(S // 128):
                pss = [fin_ps.tile([128, 512], F32, name="finps", tag="fin") for _ in range(2)]
                for p in range(2):
                    for n in range(2):
                        nc.tensor.matmul(
                            pss[n][:],
                            outTbf[p][:, m * 128:(m + 1) * 128],
                            wo[p][:, n * 512:(n + 1) * 512],
                            start=(p == 0),
                            stop=(p == 1),
                        )
                for n in range(2):
                    st = fin_sb.tile([128, 512], F32, name="finst", tag="finsb")
                    if (m + n) % 2 == 0:
                        nc.scalar.copy(st[:], pss[n][:])
                    else:
                        nc.vector.tensor_copy(st[:], pss[n][:])
                    nc.sync.dma_start(
                        outp_ap[m * 128:(m + 1) * 128, n * 512:(n + 1) * 512], st[:]
                    )


def _get_nc(kept, loop_n=0, phases=ALL_PHASES):
    key = (kept, loop_n, tuple(phases))
    if key in _nc_cache:
        return _nc_cache[key]
    nc = bacc.Bacc("TRN2", target_bir_lowering=False, debug=False, num_devices=NCORES)
    xT_ap = nc.dram_tensor("xT", [D, S], BF16, kind="ExternalInput").ap()
    wqT_ap = nc.dram_tensor("wqT", [D, E], BF16, kind="ExternalInput").ap()
    wkT_ap = nc.dram_tensor("wkT", [D, E], BF16, kind="ExternalInput").ap()
    wvT_ap = nc.dram_tensor("wvT", [D, E], BF16, kind="ExternalInput").ap()
    woT_ap = nc.dram_tensor("woT", [E, D], BF16, kind="ExternalInput").ap()
    outp_ap = nc.dram_tensor("outp", [S, D], F32, kind="ExternalOutput").ap()
    with tile.TileContext(nc) as tc:
        _emit(tc, (xT_ap, wqT_ap, wkT_ap, wvT_ap, woT_ap, outp_ap), kept,
              loop_n=loop_n, phases=phases)
    nc.compile()
    _nc_cache[key] = nc
    return nc


def kernel(x, Wq, Wk, Wv, Wo, bo, block_mask):
    x = np.asarray(x, dtype=np.float32)
    Wq = np.asarray(Wq, dtype=np.float32)
    Wk = np.asarray(Wk, dtype=np.float32)
    Wv = np.asarray(Wv, dtype=np.float32)
    Wo = np.asarray(Wo, dtype=np.float32)
    bo = np.asarray(bo, dtype=np.float32)
    mask = np.asarray(block_mask).astype(bool)

    kept = tuple(tuple(int(j) for j in np.nonzero(mask[i])[0]) for i in range(NB))
    assert all(len(js) > 0 for js in kept), "a query block row has no kept blocks"

    t0 = time.monotonic()
    nc = _get_nc(kept)
    t_compile = time.monotonic() - t0

    xT_b = [np.ascontiguousarray(x[b].T).astype(bf16) for b in range(B)]
    in_maps = []
    for c in range(NCORES):
        b = c // (NCORES // B)
        hs = c % (NCORES // B)
        sl = slice(hs * E, (hs + 1) * E)
        in_maps.append({
            "xT": xT_b[b],
            "wqT": np.ascontiguousarray((Wq[sl, :] / np.sqrt(np.float32(DH))).T).astype(bf16),
            "wkT": np.ascontiguousarray(Wk[sl, :].T).astype(bf16),
            "wvT": np.ascontiguousarray(Wv[sl, :].T).astype(bf16),
            "woT": np.ascontiguousarray(Wo[:, sl].T).astype(bf16),
        })

    t0 = time.monotonic()
    res = run_bass_kernel_spmd(nc, in_maps, list(range(NCORES)))
    t_run = time.monotonic() - t0

    out = np.zeros((B, S, D), np.float32)
    for c in range(NCORES):
        out[c // (NCORES // B)] += res.results[c]["outp"]
    out += bo[None, None, :]

    last_run_info.update(compile_s=t_compile, run_s=t_run, nc=nc)
    return out



# revision 8
# speedup vs baseline: 1.4119x; 1.4119x over previous
"""Block-sparse attention on 8 Trainium2 NeuronCores (Bass/Tile SPMD kernel).

Sharding: batch*head_groups across the 8 cores. Core c handles batch c//4 and
heads [4*(c%4), 4*(c%4)+4). Projection weights are sliced per core host-side
(pre-transposed + bf16-cast, packed so each weight is ONE dma); the [16,16]
block mask specializes the compiled program (only kept blocks are computed).
Each core emits a partial output (its 256-wide d-slice pushed through Wo) in
bf16; the host sums the 4 partials per batch in f32 and adds the bias.

Design (driven by the TimelineSim cost model, where matmul cost = N_out only):
  - scores [k, q] via stationary k^T block (K=64), moving q^T runs (N=128/blk)
  - exp on Act engine into bf16 `at` tiles (the overall pacer: ~0.83ns/elem)
  - AV *natural*: stationary = at block [128k,128q], moving = [v_h|1] [128,65]
    -> av psum [128 q, 65] accumulated over kept j (N=65/blk, den for free)
  - batched normalize: 1/den per-region (DVE reciprocal on strided AP), one
    broadcast (0-stride) tensor_tensor multiply per av bank -> norm_sb bf16
  - PE transpose (bf16 identity) norm_sb [128q,64d] -> tp psum bf16 [64,128],
    packed 2 heads x 4 blocks per [128,512]; one DVE 2x copy -> outT
  - final: outT stationary, wo moving, f32 psum -> bf16 sb -> 1 dma per m
PSUM: sc pool 2x[128,1024]f32 (4 banks, also hosts tp tiles), av 2 banks
(65-wide regions packed 7/bank per 4-i-row chunk), work pool 2 banks
(v/qk-p1 projections + final) = 8. The tile list-scheduler overlaps
projection/final PE work under the Act-bound attention automatically.
"""

import time
from contextlib import ExitStack

import ml_dtypes
import numpy as np

import concourse.bass as bass
import concourse.tile as tile
from concourse import bacc, mybir
from concourse.bass_utils import run_bass_kernel_spmd

BF16 = mybir.dt.bfloat16
F32 = mybir.dt.float32
bf16 = ml_dtypes.bfloat16

B, S, D, H = 2, 2048, 1024, 16
DH = 64
BLK = 128
NB = 16
NCORES = 8
HPC = H // (NCORES // B)   # 4 heads per core
E = HPC * DH               # 256 projection columns per core
EV = HPC * (DH + 1)        # 260: v stored as [v_h | 1] per head
KD = D // 128              # 8 contraction chunks
QC = 4                     # query-block rows per attention chunk
NQC = NB // QC             # 4 chunks
FILL = 1024                # score fill width (2 psum banks, 8 block-cols)

_nc_cache: dict = {}
last_run_info: dict = {}


def _runs_of(lst):
    out = []
    for i in lst:
        if out and i == out[-1][-1] + 1:
            out[-1].append(i)
        else:
            out.append([i])
    return out


def _emit(tc, aps, kept):
    nc = tc.nc
    xp_ap, wq_ap, wk_ap, wv_ap, wo_ap, id_ap, outp_ap = aps
    Exp = mybir.ActivationFunctionType.Exp
    first_j = {i: kept[i][0] for i in range(NB)}
    last_j = {i: kept[i][-1] for i in range(NB)}
    col_kept = [[i for i in range(NB) if j in kept[i]] for j in range(NB)]

    with ExitStack() as ctx:
        persist = ctx.enter_context(tc.tile_pool(name="persist", bufs=1))
        sc_ps = ctx.enter_context(tc.tile_pool(name="sc_ps", bufs=2, space="PSUM"))
        av_ps = ctx.enter_context(tc.tile_pool(name="av_ps", bufs=1, space="PSUM"))
        work_ps = ctx.enter_context(tc.tile_pool(name="work_ps", bufs=2, space="PSUM"))
        at_sb = ctx.enter_context(tc.tile_pool(name="at_sb", bufs=3))
        nrm_sb = ctx.enter_context(tc.tile_pool(name="nrm_sb", bufs=2))
        fin_sb = ctx.enter_context(tc.tile_pool(name="fin_sb", bufs=2))

        # ---- input loads (weights first so projections start at x arrival) --
        ident = persist.tile([128, 128], BF16, name="ident", tag="ident")
        nc.sync.dma_start(ident[:], id_ap[:, :])
        wq = persist.tile([128, 2048], BF16, name="wq", tag="wq")
        wk = persist.tile([128, 2048], BF16, name="wk", tag="wk")
        wv = persist.tile([128, 2048], BF16, name="wv", tag="wv")
        wo = persist.tile([128, 2048], BF16, name="wo", tag="wo")
        for t, ap in ((wq, wq_ap), (wk, wk_ap), (wv, wv_ap), (wo, wo_ap)):
            nc.sync.dma_start(t[:], ap[:, :])
        # x packed [part, sc, kd, s]: 4 dma chunks of one s-group each
        xT = persist.tile([128, 16384], BF16, name="xT", tag="xT")
        for g in range(4):
            nc.sync.dma_start(xT[:, g * 4096:(g + 1) * 4096],
                              xp_ap[:, g * 4096:(g + 1) * 4096])

        qT = [persist.tile([128, S], BF16, name=f"qT{p}", tag=f"qT{p}") for p in range(2)]
        kT = [persist.tile([128, S], BF16, name=f"kT{p}", tag=f"kT{p}") for p in range(2)]
        vv = [persist.tile([128, EV], BF16, name=f"v{m}", tag=f"v{m}") for m in range(NB)]
        outTbf = [persist.tile([128, S], BF16, name=f"oT{p}", tag=f"oT{p}") for p in range(2)]

        def x_mv(kd, sc):
            # moving x^T slice for contraction chunk kd, s-columns [512*sc, 512*(sc+1))
            return xT[:, sc * 4096 + kd * 512: sc * 4096 + (kd + 1) * 512]

        def proj_qk(p, on_act):
            for w, dst in ((wq, qT[p]), (wk, kT[p])):
                for sc in range(4):
                    ps = work_ps.tile([128, 512], F32, name="w_ps", tag="w")
                    for kd in range(KD):
                        nc.tensor.matmul(
                            ps[:],
                            w[:, kd * 256 + p * 128: kd * 256 + (p + 1) * 128],
                            x_mv(kd, sc),
                            start=(kd == 0), stop=(kd == KD - 1),
                        )
                    eng = nc.scalar if on_act else nc.vector
                    if on_act:
                        eng.copy(dst[:, sc * 512:(sc + 1) * 512], ps[:])
                    else:
                        eng.tensor_copy(dst[:, sc * 512:(sc + 1) * 512], ps[:])

        def proj_v():
            for m in range(NB):
                ps = work_ps.tile([128, 260], F32, name="v_ps", tag="w")
                for kd in range(KD):
                    nc.tensor.matmul(
                        ps[:, 0:E],
                        xT[:, (m // 4) * 4096 + kd * 512 + (m % 4) * 128:
                           (m // 4) * 4096 + kd * 512 + (m % 4) * 128 + 128],
                        wv[:, kd * 256:(kd + 1) * 256],
                        start=(kd == 0), stop=(kd == KD - 1),
                    )
                v3 = vv[m].rearrange("p (g c) -> p g c", g=HPC)
                nc.any.memset(v3[:, :, 64:65], 1.0)
                nc.vector.tensor_copy(
                    v3[:, :, 0:64],
                    ps[:, 0:E].rearrange("p (g c) -> p g c", g=HPC),
                )

        def attn_pair(p):
            for qc in range(NQC):
                i0 = qc * QC
                irange = list(range(i0, i0 + QC))
                # av accumulators: 8 regions (a*QC+li) of width 65: idx0-6 in
                # av0, idx7 in av1
                av0 = av_ps.tile([128, 512], F32, name="av0", tag="av0")
                av1 = av_ps.tile([128, 512], F32, name="av1", tag="av1")

                # start=True pending-zeroes the whole 2KB bank (zero region),
                # so only the FIRST matmul touching each bank may use it; all
                # other regions' first writes then read-as-zero via the mark.
                primed = [False, False]

                def av_region(a, li):
                    idx = a * QC + li
                    bank = av0 if idx < 7 else av1
                    c = 65 * (idx % 7)
                    return idx >= 7, bank[:, c:c + 65]

                for a in range(2):
                    h = 2 * p + a
                    rows = slice(a * 64, (a + 1) * 64)

                    def flush(groups, sc_t, colw):
                        at = at_sb.tile([128, FILL], BF16, name="at", tag="at")
                        nc.scalar.activation(at[:, 0:colw], sc_t[:, 0:colw], Exp)
                        for j, ks, off in groups:
                            lhs_v = vv[j][:, 65 * h:65 * h + 65]
                            for idx, i in enumerate(ks):
                                bank_id, region = av_region(a, i - i0)
                                nc.tensor.matmul(
                                    region,
                                    at[:, off + idx * 128: off + (idx + 1) * 128],
                                    lhs_v,
                                    start=not primed[bank_id],
                                    stop=(j == last_j[i]),
                                    skip_group_check=True,
                                )
                                primed[bank_id] = True

                    groups = []
                    sc_t = None
                    colw = 0
                    for j in range(NB):
                        ks = [i for i in col_kept[j] if i in irange]
                        if not ks:
                            continue
                        n = len(ks)
                        if sc_t is None or colw + n * 128 > FILL:
                            if sc_t is not None:
                                flush(groups, sc_t, colw)
                            sc_t = sc_ps.tile([128, FILL], F32, name="sc", tag="sc")
                            groups = []
                            colw = 0
                        for run in _runs_of(ks):
                            idx0 = ks.index(run[0])
                            col = colw + idx0 * 128
                            width = len(run) * 128
                            qcol = run[0] * 128
                            done = 0
                            while done < width:
                                seg = min(width - done, 512 - ((col + done) % 512))
                                nc.tensor.matmul(
                                    sc_t[:, col + done: col + done + seg],
                                    kT[p][rows, j * 128:(j + 1) * 128],
                                    qT[p][rows, qcol + done: qcol + done + seg],
                                )
                                done += seg
                        groups.append((j, ks, colw))
                        colw += n * 128
                    if sc_t is not None:
                        flush(groups, sc_t, colw)

                # ---- normalize + transpose this chunk ----
                rc = nrm_sb.tile([128, 8], F32, name="rc", tag="rc")
                nc.vector.reciprocal(rc[:, 0:7], av0[:, 64::65])
                nc.vector.reciprocal(rc[:, 7:8], av1[:, 64:65])
                nrm = nrm_sb.tile([128, 512], BF16, name="nrm", tag="nrm")
                nc.vector.tensor_tensor(
                    nrm[:, 0:448].rearrange("p (r c) -> p r c", c=64),
                    av0[:, 0:455].rearrange("p (r c) -> p r c", c=65)[:, :, 0:64],
                    rc[:, 0:7].unsqueeze(2).broadcast_to([128, 7, 64]),
                    mybir.AluOpType.mult,
                )
                nc.vector.tensor_tensor(
                    nrm[:, 448:512],
                    av1[:, 0:64],
                    rc[:, 7:8].broadcast_to([128, 64]),
                    mybir.AluOpType.mult,
                )
                tp = sc_ps.tile([128, 512], BF16, name="tp", tag="sc")
                for a in range(2):
                    for li in range(QC):
                        idx = a * QC + li
                        nc.tensor.transpose(
                            tp[a * 64:(a + 1) * 64, li * 128:(li + 1) * 128],
                            nrm[:, idx * 64:(idx + 1) * 64],
                            ident[:],
                        )
                nc.vector.tensor_copy(
                    outTbf[p][:, qc * 512:(qc + 1) * 512], tp[:])

        def final():
            for m in range(NB):
                fsb = fin_sb.tile([128, 1024], BF16, name="fsb", tag="fsb")
                for n in range(2):
                    ps = work_ps.tile([128, 512], F32, name="f_ps", tag="w")
                    for p in range(2):
                        nc.tensor.matmul(
                            ps[:],
                            outTbf[p][:, m * 128:(m + 1) * 128],
                            wo[:, p * 1024 + n * 512: p * 1024 + (n + 1) * 512],
                            start=(p == 0), stop=(p == 1),
                        )
                    nc.vector.tensor_copy(fsb[:, n * 512:(n + 1) * 512], ps[:])
                nc.sync.dma_start(
                    outp_ap[m * 128:(m + 1) * 128, :], fsb[:])

        # emission order = scheduler priority: qk p0 first (attention p0 can
        # start), then v, attention p0, qk p1 + attention p1, final. The list
        # scheduler runs lower-priority proj/final work whenever attention
        # stalls on the activation engine.
        proj_qk(0, on_act=True)
        proj_v()
        attn_pair(0)
        proj_qk(1, on_act=False)
        attn_pair(1)
        final()


def _get_nc(kept):
    key = kept
    if key in _nc_cache:
        return _nc_cache[key]
    nc = bacc.Bacc("TRN2", target_bir_lowering=False, debug=False, num_devices=NCORES)
    xp_ap = nc.dram_tensor("xp", [128, 16384], BF16, kind="ExternalInput").ap()
    wq_ap = nc.dram_tensor("wqp", [128, 2048], BF16, kind="ExternalInput").ap()
    wk_ap = nc.dram_tensor("wkp", [128, 2048], BF16, kind="ExternalInput").ap()
    wv_ap = nc.dram_tensor("wvp", [128, 2048], BF16, kind="ExternalInput").ap()
    wo_ap = nc.dram_tensor("wop", [128, 2048], BF16, kind="ExternalInput").ap()
    id_ap = nc.dram_tensor("ident", [128, 128], BF16, kind="ExternalInput").ap()
    outp_ap = nc.dram_tensor("outp", [S, D], BF16, kind="ExternalOutput").ap()
    with tile.TileContext(nc) as tc:
        _emit(tc, (xp_ap, wq_ap, wk_ap, wv_ap, wo_ap, id_ap, outp_ap), kept)
    nc.compile()
    _nc_cache[key] = nc
    return nc


def _pack_x(xb):
    # x[b].T [1024,2048] -> [part, sc, kd, s-within] -> [128, 16384]
    t = np.ascontiguousarray(xb.T).astype(bf16)          # [1024, 2048]
    t = t.reshape(KD, 128, 4, 512).transpose(1, 2, 0, 3)  # [128, 4, 8, 512]
    return np.ascontiguousarray(t.reshape(128, 16384))


def _pack_w(wslT):
    # W[sl,:].T [1024, 256] -> [128, kd*256]
    t = wslT.reshape(KD, 128, 256).transpose(1, 0, 2)
    return np.ascontiguousarray(t.reshape(128, 2048)).astype(bf16)


def _pack_wo(woT):
    # Wo[:,sl].T [256, 1024] -> [128, p*1024 + outcol]
    t = woT.reshape(2, 128, 1024).transpose(1, 0, 2)
    return np.ascontiguousarray(t.reshape(128, 2048)).astype(bf16)


def kernel(x, Wq, Wk, Wv, Wo, bo, block_mask):
    x = np.asarray(x, dtype=np.float32)
    Wq = np.asarray(Wq, dtype=np.float32)
    Wk = np.asarray(Wk, dtype=np.float32)
    Wv = np.asarray(Wv, dtype=np.float32)
    Wo = np.asarray(Wo, dtype=np.float32)
    bo = np.asarray(bo, dtype=np.float32)
    mask = np.asarray(block_mask).astype(bool)

    kept = tuple(tuple(int(j) for j in np.nonzero(mask[i])[0]) for i in range(NB))
    assert all(len(js) > 0 for js in kept), "a query block row has no kept blocks"

    t0 = time.monotonic()
    nc = _get_nc(kept)
    t_compile = time.monotonic() - t0

    ident = np.eye(128).astype(bf16)
    xp_b = [_pack_x(x[b]) for b in range(B)]
    in_maps = []
    for c in range(NCORES):
        b = c // (NCORES // B)
        hs = c % (NCORES // B)
        sl = slice(hs * E, (hs + 1) * E)
        in_maps.append({
            "xp": xp_b[b],
            "wqp": _pack_w(np.ascontiguousarray(
                (Wq[sl, :] / np.sqrt(np.float32(DH))).T).astype(np.float32)),
            "wkp": _pack_w(np.ascontiguousarray(Wk[sl, :].T).astype(np.float32)),
            "wvp": _pack_w(np.ascontiguousarray(Wv[sl, :].T).astype(np.float32)),
            "wop": _pack_wo(np.ascontiguousarray(Wo[:, sl].T).astype(np.float32)),
            "ident": ident,
        })

    t0 = time.monotonic()
    res = run_bass_kernel_spmd(nc, in_maps, list(range(NCORES)))
    t_run = time.monotonic() - t0

    out = np.zeros((B, S, D), np.float32)
    for c in range(NCORES):
        out[c // (NCORES // B)] += res.results[c]["outp"].astype(np.float32)
    out += bo[None, None, :]

    last_run_info.update(compile_s=t_compile, run_s=t_run, nc=nc)
    return out


# revision 40
# speedup vs baseline: 1.6374x; 1.1597x over previous
"""Block-sparse attention on 8 Trainium2 NeuronCores (Bass/Tile SPMD kernel).

Sharding: batch*head_groups across the 8 cores. Core c handles batch c//4 and
heads [4*(c%4), 4*(c%4)+4). Projection weights are sliced per core host-side
(pre-transposed + bf16-cast, packed so each weight is ONE dma); the [16,16]
block mask specializes the compiled program (only kept blocks are computed).
Each core emits a partial output (its 256-wide d-slice pushed through Wo) in
bf16; the host sums the 4 partials per batch in f32 and adds the bias.

Design (driven by the TimelineSim cost model, where matmul cost = N_out only):
  - scores [k, q] via stationary k^T block (K=64), moving q^T runs (N=128/blk)
  - exp on Act engine into bf16 `at` tiles (the overall pacer: ~0.83ns/elem)
  - AV *natural*: stationary = at block [128k,128q], moving = [v_h|1] [128,65]
    -> av psum [128 q, 65] accumulated over kept j (N=65/blk, den for free)
  - batched normalize: 1/den per-region (DVE reciprocal on strided AP), one
    broadcast (0-stride) tensor_tensor multiply per av bank -> norm_sb bf16
  - PE transpose (bf16 identity) norm_sb [128q,64d] -> tp psum bf16 [64,128],
    packed 2 heads x 4 blocks per [128,512]; one DVE 2x copy -> outT
  - final: outT stationary, wo moving, f32 psum -> bf16 sb -> 1 dma per m
PSUM: sc pool 2x[128,1024]f32 (4 banks, also hosts tp tiles), av 2 banks
(65-wide regions packed 7/bank per 4-i-row chunk), work pool 2 banks
(v/qk-p1 projections + final) = 8. The tile list-scheduler overlaps
projection/final PE work under the Act-bound attention automatically.
"""

import time
from contextlib import ExitStack

import ml_dtypes
import numpy as np

import concourse.bass as bass
import concourse.tile as tile
from concourse import bacc, mybir
from concourse.bass_utils import run_bass_kernel_spmd

BF16 = mybir.dt.bfloat16
F32 = mybir.dt.float32
bf16 = ml_dtypes.bfloat16

B, S, D, H = 2, 2048, 1024, 16
DH = 64
BLK = 128
NB = 16
NCORES = 8
HPC = H // (NCORES // B)   # 4 heads per core
E = HPC * DH               # 256 projection columns per core
EV = HPC * (DH + 1)        # 260: v stored as [v_h | 1] per head
KD = D // 128              # 8 contraction chunks
QC = 4                     # query-block rows per attention chunk
NQC = NB // QC             # 4 chunks
FILL = 1024                # score fill width (2 psum banks, 8 block-cols)

_nc_cache: dict = {}
last_run_info: dict = {}
AT_BUFS = 28
TP_IN_WORK = False

# NOTE: emission order IS program order for the tile dep tracker — every
# tensor's writer must be emitted before its readers (v before all attention,
# qk1 before a1*, Fg after both g-chunks). high_priority on attention makes
# the low-priority proj/final work execute lazily as PE gap filler anyway.
ORDER = ("qk0", "v", "a00", "a01", "a02", "a03", "qk1", "a10",
         "F0", "a11", "F1", "a12", "F2", "a13", "F3t")


def _order():
    return ORDER


def _runs_of(lst):
    out = []
    for i in lst:
        if out and i == out[-1][-1] + 1:
            out[-1].append(i)
        else:
            out.append([i])
    return out


def _emit(tc, aps, kept):
    nc = tc.nc
    xp_ap, wq_ap, wk_ap, wv_ap, wo_ap, id_ap, outp_ap = aps
    Exp = mybir.ActivationFunctionType.Exp
    first_j = {i: kept[i][0] for i in range(NB)}
    last_j = {i: kept[i][-1] for i in range(NB)}
    col_kept = [[i for i in range(NB) if j in kept[i]] for j in range(NB)]

    with ExitStack() as ctx:
        persist = ctx.enter_context(tc.tile_pool(name="persist", bufs=1))
        sc_ps = ctx.enter_context(tc.tile_pool(name="sc_ps", bufs=2, space="PSUM"))
        work_ps = ctx.enter_context(tc.tile_pool(name="work_ps", bufs=2, space="PSUM"))
        av_ps = ctx.enter_context(tc.tile_pool(name="av_ps", bufs=1, space="PSUM"))
        at_sb = ctx.enter_context(tc.tile_pool(name="at_sb", bufs=AT_BUFS))
        nrm_sb = ctx.enter_context(tc.tile_pool(name="nrm_sb", bufs=4))
        fin_sb = ctx.enter_context(tc.tile_pool(name="fin_sb", bufs=4))

        # ---- input loads: wq then x0 first so q-proj sc0 starts earliest ----
        ident = persist.tile([128, 128], BF16, name="ident", tag="ident")
        wq = persist.tile([128, 2048], BF16, name="wq", tag="wq")
        wk = persist.tile([128, 2048], BF16, name="wk", tag="wk")
        wv = persist.tile([128, 2048], BF16, name="wv", tag="wv")
        wo = persist.tile([128, 2048], BF16, name="wo", tag="wo")
        # x packed [part, sc, kd, s]: one tile per s-group so deps are exact
        xg = [persist.tile([128, 4096], BF16, name=f"xg{g}", tag=f"xg{g}")
              for g in range(4)]
        nc.sync.dma_start(wq[:], wq_ap[:, :])
        nc.sync.dma_start(xg[0][:, 0:2048], xp_ap[:, 0:2048])
        nc.sync.dma_start(xg[0][:, 2048:4096], xp_ap[:, 2048:4096])
        nc.sync.dma_start(wk[:], wk_ap[:, :])
        nc.sync.dma_start(xg[1][:], xp_ap[:, 4096:8192])
        nc.sync.dma_start(wv[:], wv_ap[:, :])
        nc.sync.dma_start(xg[2][:], xp_ap[:, 8192:12288])
        nc.sync.dma_start(wo[:], wo_ap[:, :])
        nc.sync.dma_start(ident[:], id_ap[:, :])
        nc.sync.dma_start(xg[3][:], xp_ap[:, 12288:16384])

        qT = [persist.tile([128, S], BF16, name=f"qT{p}", tag=f"qT{p}") for p in range(2)]
        kT = [persist.tile([128, S], BF16, name=f"kT{p}", tag=f"kT{p}") for p in range(2)]
        vv = [persist.tile([128, EV], BF16, name=f"v{m}", tag=f"v{m}") for m in range(NB)]
        outTbf = [persist.tile([128, S], BF16, name=f"oT{p}", tag=f"oT{p}") for p in range(2)]

        def x_mv(kd, sc):
            # moving x^T slice for contraction chunk kd, s-columns [512*sc, 512*(sc+1))
            return xg[sc][:, kd * 512:(kd + 1) * 512]

        def proj_qk(p):
            # sc outer so each x-group is fully consumed as it arrives off DMA
            for sc in range(4):
                for w, dst in ((wq, qT[p]), (wk, kT[p])):
                    ps = work_ps.tile([128, 512], F32, name="w_ps", tag="w")
                    for kd in range(KD):
                        nc.tensor.matmul(
                            ps[:],
                            w[:, kd * 256 + p * 128: kd * 256 + (p + 1) * 128],
                            x_mv(kd, sc),
                            start=(kd == 0), stop=(kd == KD - 1),
                        )
                    nc.vector.tensor_copy(dst[:, sc * 512:(sc + 1) * 512], ps[:])

        def proj_v(ms=range(NB)):
            for m in ms:
                ps = work_ps.tile([128, 260], F32, name="v_ps", tag="w")
                for kd in range(KD):
                    nc.tensor.matmul(
                        ps[:, 0:E],
                        xg[m // 4][:, kd * 512 + (m % 4) * 128:
                                   kd * 512 + (m % 4) * 128 + 128],
                        wv[:, kd * 256:(kd + 1) * 256],
                        start=(kd == 0), stop=(kd == KD - 1),
                    )
                v3 = vv[m].rearrange("p (g c) -> p g c", g=HPC)
                nc.any.memset(v3[:, :, 64:65], 1.0)
                nc.vector.tensor_copy(
                    v3[:, :, 0:64],
                    ps[:, 0:E].rearrange("p (g c) -> p g c", g=HPC),
                )

        def attn_chunk(p, qc):
            # attention always outranks proj/final filler in the scheduler's
            # ready heap (dependencies still force projections to run first
            # where needed); emission order still controls psum slot FIFO.
            with tc.high_priority(offset=1_000_000):
                i0 = qc * QC
                irange = list(range(i0, i0 + QC))
                # tp allocated FIRST so the sc-pool slot rotation never makes a
                # later QK flush wait on this chunk's tp copy
                if TP_IN_WORK:
                    tp = work_ps.tile([128, 512], BF16, name="tp", tag="w")
                else:
                    tp = sc_ps.tile([128, 512], BF16, name="tp", tag="sc")
                # av accumulators: one bank per head, 4 regions of width 65
                av = [av_ps.tile([128, 512], F32, name=f"av{a}", tag=f"av{a}")
                      for a in range(2)]

                # start=True pending-zeroes the whole 2KB bank (zero region),
                # so only the FIRST matmul touching each bank may use it; all
                # other regions' first writes then read-as-zero via the mark.
                primed = [False, False]

                def av_region(a, li):
                    return a, av[a][:, 65 * li:65 * li + 65]

                for a in range(2):
                    h = 2 * p + a
                    rows = slice(a * 64, (a + 1) * 64)

                    def flush(groups, sc_t, colw):
                        at = at_sb.tile([128, FILL], BF16, name="at", tag="at")
                        nc.scalar.activation(at[:, 0:colw], sc_t[:, 0:colw], Exp)
                        for j, ks, off in groups:
                            lhs_v = vv[j][:, 65 * h:65 * h + 65]
                            for idx, i in enumerate(ks):
                                bank_id, region = av_region(a, i - i0)
                                nc.tensor.matmul(
                                    region,
                                    at[:, off + idx * 128: off + (idx + 1) * 128],
                                    lhs_v,
                                    start=not primed[bank_id],
                                    stop=(j == last_j[i]),
                                    skip_group_check=True,
                                )
                                primed[bank_id] = True

                    groups = []
                    sc_t = None
                    colw = 0
                    for j in range(NB):
                        ks = [i for i in col_kept[j] if i in irange]
                        if not ks:
                            continue
                        n = len(ks)
                        if sc_t is None or colw + n * 128 > FILL:
                            if sc_t is not None:
                                flush(groups, sc_t, colw)
                            sc_t = sc_ps.tile([128, FILL], F32, name="sc", tag="sc")
                            groups = []
                            colw = 0
                        for run in _runs_of(ks):
                            idx0 = ks.index(run[0])
                            col = colw + idx0 * 128
                            width = len(run) * 128
                            qcol = run[0] * 128
                            done = 0
                            while done < width:
                                seg = min(width - done, 512 - ((col + done) % 512))
                                nc.tensor.matmul(
                                    sc_t[:, col + done: col + done + seg],
                                    kT[p][rows, j * 128:(j + 1) * 128],
                                    qT[p][rows, qcol + done: qcol + done + seg],
                                )
                                done += seg
                        groups.append((j, ks, colw))
                        colw += n * 128
                    if sc_t is not None:
                        flush(groups, sc_t, colw)

                # ---- normalize + transpose this chunk (per head so head A's
                # transposes unlock while head B's normalize runs) ----
                rc = nrm_sb.tile([128, 8], F32, name="rc", tag="rc")
                nrm = nrm_sb.tile([128, 512], BF16, name="nrm", tag="nrm")
                for a in range(2):
                    nc.vector.reciprocal(rc[:, 4 * a:4 * a + 4], av[a][:, 64:260:65])
                    nc.vector.tensor_tensor(
                        nrm[:, a * 256:a * 256 + 256].rearrange("p (r c) -> p r c", c=64),
                        av[a][:, 0:260].rearrange("p (r c) -> p r c", c=65)[:, :, 0:64],
                        rc[:, 4 * a:4 * a + 4].unsqueeze(2).broadcast_to([128, 4, 64]),
                        mybir.AluOpType.mult,
                    )
                    for li in range(QC):
                        idx = a * QC + li
                        nc.tensor.transpose(
                            tp[a * 64:(a + 1) * 64, li * 128:(li + 1) * 128],
                            nrm[:, idx * 64:(idx + 1) * 64],
                            ident[:],
                        )
                nc.vector.tensor_copy(
                    outTbf[p][:, qc * 512:(qc + 1) * 512], tp[:])

        def final(ms, tail=False):
            # mid-attention groups: copies on DVE only (Act must stay free for
            # exp); the post-attention tail group splits copies across engines
            # and borrows the idle av-pool banks for more psum parallelism
            for mi, m in enumerate(ms):
                fsb = fin_sb.tile([128, 1024], BF16, name="fsb", tag="fsb")
                for n in range(2):
                    if tail and (2 * mi + n) % 2 == 1:
                        ps = av_ps.tile([128, 512], F32, name="f_av",
                                        tag=f"av{(2 * mi + n) // 2 % 2}")
                    else:
                        ps = work_ps.tile([128, 512], F32, name="f_ps", tag="w")
                    for p in range(2):
                        nc.tensor.matmul(
                            ps[:],
                            outTbf[p][:, m * 128:(m + 1) * 128],
                            wo[:, p * 1024 + n * 512: p * 1024 + (n + 1) * 512],
                            start=(p == 0), stop=(p == 1),
                        )
                    if tail and n == 1:
                        nc.scalar.copy(fsb[:, n * 512:(n + 1) * 512], ps[:])
                    else:
                        nc.vector.tensor_copy(fsb[:, n * 512:(n + 1) * 512], ps[:])
                nc.sync.dma_start(
                    outp_ap[m * 128:(m + 1) * 128, :], fsb[:])

        # emission order = scheduler priority. Interleave p0/p1 chunks and emit
        # each final m-group right after the (p1,qc) that completes its outT
        # columns, so final matmuls act as PE filler while later chunks stall
        # on the activation engine (exp).
        # Emission order = scheduler priority: attention QK outranks the bulk
        # projections (v, qk p1) so the Act engine is fed scores ASAP; the
        # lower-priority projections + final groups then fill PE stalls.
        # emission order = scheduler priority AND psum slot FIFO order; see
        # ORDER spec tokens: qk0/qk1, vA (m0-7), vB (m8-15), aPQ, F0-F2, F3t
        for tok in _order():
            if tok.startswith("#"):
                continue
            if tok == "qk0":
                proj_qk(0)
            elif tok == "qk1":
                proj_qk(1)
            elif tok == "v":
                proj_v()
            elif tok == "vA":
                proj_v(range(0, 8))
            elif tok == "vB":
                proj_v(range(8, 16))
            elif tok.startswith("a"):
                attn_chunk(int(tok[1]), int(tok[2]))
            elif tok.startswith("F"):
                g = int(tok[1])
                final([4 * g + k for k in range(4)], tail=tok.endswith("t"))
            else:
                raise ValueError(tok)


def _get_nc(kept):
    key = (kept, ORDER)
    if key in _nc_cache:
        return _nc_cache[key]
    nc = bacc.Bacc("TRN2", target_bir_lowering=False, debug=False, num_devices=NCORES)
    xp_ap = nc.dram_tensor("xp", [128, 16384], BF16, kind="ExternalInput").ap()
    wq_ap = nc.dram_tensor("wqp", [128, 2048], BF16, kind="ExternalInput").ap()
    wk_ap = nc.dram_tensor("wkp", [128, 2048], BF16, kind="ExternalInput").ap()
    wv_ap = nc.dram_tensor("wvp", [128, 2048], BF16, kind="ExternalInput").ap()
    wo_ap = nc.dram_tensor("wop", [128, 2048], BF16, kind="ExternalInput").ap()
    id_ap = nc.dram_tensor("ident", [128, 128], BF16, kind="ExternalInput").ap()
    outp_ap = nc.dram_tensor("outp", [S, D], BF16, kind="ExternalOutput").ap()
    with tile.TileContext(nc) as tc:
        _emit(tc, (xp_ap, wq_ap, wk_ap, wv_ap, wo_ap, id_ap, outp_ap), kept)
    nc.compile()
    _nc_cache[key] = nc
    return nc


def _pack_x(xb):
    # x[b].T [1024,2048] -> [part, sc, kd, s-within] -> [128, 16384]
    t = np.ascontiguousarray(xb.T).astype(bf16)          # [1024, 2048]
    t = t.reshape(KD, 128, 4, 512).transpose(1, 2, 0, 3)  # [128, 4, 8, 512]
    return np.ascontiguousarray(t.reshape(128, 16384))


def _pack_w(wslT):
    # W[sl,:].T [1024, 256] -> [128, kd*256]
    t = wslT.reshape(KD, 128, 256).transpose(1, 0, 2)
    return np.ascontiguousarray(t.reshape(128, 2048)).astype(bf16)


def _pack_wo(woT):
    # Wo[:,sl].T [256, 1024] -> [128, p*1024 + outcol]
    t = woT.reshape(2, 128, 1024).transpose(1, 0, 2)
    return np.ascontiguousarray(t.reshape(128, 2048)).astype(bf16)


def kernel(x, Wq, Wk, Wv, Wo, bo, block_mask):
    x = np.asarray(x, dtype=np.float32)
    Wq = np.asarray(Wq, dtype=np.float32)
    Wk = np.asarray(Wk, dtype=np.float32)
    Wv = np.asarray(Wv, dtype=np.float32)
    Wo = np.asarray(Wo, dtype=np.float32)
    bo = np.asarray(bo, dtype=np.float32)
    mask = np.asarray(block_mask).astype(bool)

    kept = tuple(tuple(int(j) for j in np.nonzero(mask[i])[0]) for i in range(NB))
    assert all(len(js) > 0 for js in kept), "a query block row has no kept blocks"

    t0 = time.monotonic()
    nc = _get_nc(kept)
    t_compile = time.monotonic() - t0

    ident = np.eye(128).astype(bf16)
    xp_b = [_pack_x(x[b]) for b in range(B)]
    in_maps = []
    for c in range(NCORES):
        b = c // (NCORES // B)
        hs = c % (NCORES // B)
        sl = slice(hs * E, (hs + 1) * E)
        in_maps.append({
            "xp": xp_b[b],
            "wqp": _pack_w(np.ascontiguousarray(
                (Wq[sl, :] / np.sqrt(np.float32(DH))).T).astype(np.float32)),
            "wkp": _pack_w(np.ascontiguousarray(Wk[sl, :].T).astype(np.float32)),
            "wvp": _pack_w(np.ascontiguousarray(Wv[sl, :].T).astype(np.float32)),
            "wop": _pack_wo(np.ascontiguousarray(Wo[:, sl].T).astype(np.float32)),
            "ident": ident,
        })

    t0 = time.monotonic()
    res = run_bass_kernel_spmd(nc, in_maps, list(range(NCORES)))
    t_run = time.monotonic() - t0

    out = np.zeros((B, S, D), np.float32)
    for c in range(NCORES):
        out[c // (NCORES // B)] += res.results[c]["outp"].astype(np.float32)
    out += bo[None, None, :]

    last_run_info.update(compile_s=t_compile, run_s=t_run, nc=nc)
    return out


# revision 42
# speedup vs baseline: 1.7299x; 1.0565x over previous
"""Block-sparse attention on 8 Trainium2 NeuronCores (Bass/Tile SPMD kernel).

Sharding: batch*head_groups across the 8 cores. Core c handles batch c//4 and
heads [4*(c%4), 4*(c%4)+4). Projection weights are sliced per core host-side
(pre-transposed + bf16-cast, packed so each weight is ONE dma); the [16,16]
block mask specializes the compiled program (only kept blocks are computed).
Each core emits a partial output (its 256-wide d-slice pushed through Wo) in
bf16; the host sums the 4 partials per batch in f32 and adds the bias.

Design (driven by the TimelineSim cost model, where matmul cost = N_out only):
  - scores [k, q] via stationary k^T block (K=64), moving q^T runs (N=128/blk)
  - exp on Act engine into bf16 `at` tiles (the overall pacer: ~0.83ns/elem)
  - AV *natural*: stationary = at block [128k,128q], moving = [v_h|1] [128,65]
    -> av psum [128 q, 65] accumulated over kept j (N=65/blk, den for free)
  - batched normalize: 1/den per-region (DVE reciprocal on strided AP), one
    broadcast (0-stride) tensor_tensor multiply per av bank -> norm_sb bf16
  - PE transpose (bf16 identity) norm_sb [128q,64d] -> tp psum bf16 [64,128],
    packed 2 heads x 4 blocks per [128,512]; one DVE 2x copy -> outT
  - final: outT stationary, wo moving, f32 psum -> bf16 sb -> 1 dma per m
PSUM: sc pool 2x[128,1024]f32 (4 banks, also hosts tp tiles), av 2 banks
(65-wide regions packed 7/bank per 4-i-row chunk), work pool 2 banks
(v/qk-p1 projections + final) = 8. The tile list-scheduler overlaps
projection/final PE work under the Act-bound attention automatically.
"""

import time
from contextlib import ExitStack

import ml_dtypes
import numpy as np

import concourse.bass as bass
import concourse.tile as tile
from concourse import bacc, mybir
from concourse.bass_utils import run_bass_kernel_spmd

BF16 = mybir.dt.bfloat16
F32 = mybir.dt.float32
bf16 = ml_dtypes.bfloat16

B, S, D, H = 2, 2048, 1024, 16
DH = 64
BLK = 128
NB = 16
NCORES = 8
HPC = H // (NCORES // B)   # 4 heads per core
E = HPC * DH               # 256 projection columns per core
EV = HPC * (DH + 1)        # 260: v stored as [v_h | 1] per head
KD = D // 128              # 8 contraction chunks
QC = 4                     # query-block rows per attention chunk
NQC = NB // QC             # 4 chunks
FILL = 1024                # score fill width (2 psum banks, 8 block-cols)

_nc_cache: dict = {}
last_run_info: dict = {}
AT_BUFS = 28
TP_IN_WORK = True
V_PRIO = 0
TP_AT_END = False

# NOTE: emission order IS program order for the tile dep tracker — every
# tensor's writer must be emitted before its readers (v before all attention,
# qk1 before a1*, Fg after both g-chunks). high_priority on attention makes
# the low-priority proj/final work execute lazily as PE gap filler anyway.
ORDER = ("qk0", "v", "a00", "a01", "a02", "a03", "qk1", "a10",
         "F0", "a11", "F1", "a12", "F2", "a13", "F3t")


def _order():
    return ORDER


def _runs_of(lst):
    out = []
    for i in lst:
        if out and i == out[-1][-1] + 1:
            out[-1].append(i)
        else:
            out.append([i])
    return out


def _emit(tc, aps, kept):
    nc = tc.nc
    xp_ap, wq_ap, wk_ap, wv_ap, wo_ap, id_ap, outp_ap = aps
    Exp = mybir.ActivationFunctionType.Exp
    first_j = {i: kept[i][0] for i in range(NB)}
    last_j = {i: kept[i][-1] for i in range(NB)}
    col_kept = [[i for i in range(NB) if j in kept[i]] for j in range(NB)]

    with ExitStack() as ctx:
        persist = ctx.enter_context(tc.tile_pool(name="persist", bufs=1))
        sc_ps = ctx.enter_context(tc.tile_pool(name="sc_ps", bufs=2, space="PSUM"))
        work_ps = ctx.enter_context(tc.tile_pool(name="work_ps", bufs=2, space="PSUM"))
        av_ps = ctx.enter_context(tc.tile_pool(name="av_ps", bufs=1, space="PSUM"))
        at_sb = ctx.enter_context(tc.tile_pool(name="at_sb", bufs=AT_BUFS))
        nrm_sb = ctx.enter_context(tc.tile_pool(name="nrm_sb", bufs=4))
        fin_sb = ctx.enter_context(tc.tile_pool(name="fin_sb", bufs=4))

        # ---- input loads: wq then x0 first so q-proj sc0 starts earliest ----
        ident = persist.tile([128, 128], BF16, name="ident", tag="ident")
        wq = persist.tile([128, 2048], BF16, name="wq", tag="wq")
        wk = persist.tile([128, 2048], BF16, name="wk", tag="wk")
        wv = persist.tile([128, 2048], BF16, name="wv", tag="wv")
        wo = persist.tile([128, 2048], BF16, name="wo", tag="wo")
        # x packed [part, sc, kd, s]: one tile per s-group so deps are exact
        xg = [persist.tile([128, 4096], BF16, name=f"xg{g}", tag=f"xg{g}")
              for g in range(4)]
        nc.sync.dma_start(wq[:], wq_ap[:, :])
        nc.sync.dma_start(xg[0][:, 0:2048], xp_ap[:, 0:2048])
        nc.sync.dma_start(xg[0][:, 2048:4096], xp_ap[:, 2048:4096])
        nc.sync.dma_start(wk[:], wk_ap[:, :])
        nc.sync.dma_start(xg[1][:], xp_ap[:, 4096:8192])
        nc.sync.dma_start(wv[:], wv_ap[:, :])
        nc.sync.dma_start(xg[2][:], xp_ap[:, 8192:12288])
        nc.sync.dma_start(wo[:], wo_ap[:, :])
        nc.sync.dma_start(ident[:], id_ap[:, :])
        nc.sync.dma_start(xg[3][:], xp_ap[:, 12288:16384])

        qT = [persist.tile([128, S], BF16, name=f"qT{p}", tag=f"qT{p}") for p in range(2)]
        kT = [persist.tile([128, S], BF16, name=f"kT{p}", tag=f"kT{p}") for p in range(2)]
        vv = [persist.tile([128, EV], BF16, name=f"v{m}", tag=f"v{m}") for m in range(NB)]
        outTbf = [persist.tile([128, S], BF16, name=f"oT{p}", tag=f"oT{p}") for p in range(2)]

        def x_mv(kd, sc):
            # moving x^T slice for contraction chunk kd, s-columns [512*sc, 512*(sc+1))
            return xg[sc][:, kd * 512:(kd + 1) * 512]

        def proj_qk(p):
            # sc outer so each x-group is fully consumed as it arrives off DMA
            for sc in range(4):
                for w, dst in ((wq, qT[p]), (wk, kT[p])):
                    ps = work_ps.tile([128, 512], F32, name="w_ps", tag="w")
                    for kd in range(KD):
                        nc.tensor.matmul(
                            ps[:],
                            w[:, kd * 256 + p * 128: kd * 256 + (p + 1) * 128],
                            x_mv(kd, sc),
                            start=(kd == 0), stop=(kd == KD - 1),
                        )
                    nc.vector.tensor_copy(dst[:, sc * 512:(sc + 1) * 512], ps[:])

        def proj_v(ms=range(NB)):
            from contextlib import nullcontext
            pctx = tc.high_priority(offset=V_PRIO) if V_PRIO else nullcontext()
            with pctx:
                _proj_v_body(ms)

        def _proj_v_body(ms):
            for m in ms:
                ps = work_ps.tile([128, 260], F32, name="v_ps", tag="w")
                for kd in range(KD):
                    nc.tensor.matmul(
                        ps[:, 0:E],
                        xg[m // 4][:, kd * 512 + (m % 4) * 128:
                                   kd * 512 + (m % 4) * 128 + 128],
                        wv[:, kd * 256:(kd + 1) * 256],
                        start=(kd == 0), stop=(kd == KD - 1),
                    )
                v3 = vv[m].rearrange("p (g c) -> p g c", g=HPC)
                nc.any.memset(v3[:, :, 64:65], 1.0)
                nc.vector.tensor_copy(
                    v3[:, :, 0:64],
                    ps[:, 0:E].rearrange("p (g c) -> p g c", g=HPC),
                )

        def attn_chunk(p, qc):
            # attention always outranks proj/final filler in the scheduler's
            # ready heap (dependencies still force projections to run first
            # where needed); emission order still controls psum slot FIFO.
            with tc.high_priority(offset=1_000_000):
                i0 = qc * QC
                irange = list(range(i0, i0 + QC))
                # tp allocated FIRST so the sc-pool slot rotation never makes a
                # later QK flush wait on this chunk's tp copy
                def alloc_tp():
                    if TP_IN_WORK:
                        return work_ps.tile([128, 512], BF16, name="tp", tag="w")
                    return sc_ps.tile([128, 512], BF16, name="tp", tag="sc")
                tp = None if TP_AT_END else alloc_tp()
                # av accumulators: one bank per head, 4 regions of width 65
                av = [av_ps.tile([128, 512], F32, name=f"av{a}", tag=f"av{a}")
                      for a in range(2)]

                # start=True pending-zeroes the whole 2KB bank (zero region),
                # so only the FIRST matmul touching each bank may use it; all
                # other regions' first writes then read-as-zero via the mark.
                primed = [False, False]

                def av_region(a, li):
                    return a, av[a][:, 65 * li:65 * li + 65]

                for a in range(2):
                    h = 2 * p + a
                    rows = slice(a * 64, (a + 1) * 64)

                    def flush(groups, sc_t, colw):
                        at = at_sb.tile([128, FILL], BF16, name="at", tag="at")
                        nc.scalar.activation(at[:, 0:colw], sc_t[:, 0:colw], Exp)
                        for j, ks, off in groups:
                            lhs_v = vv[j][:, 65 * h:65 * h + 65]
                            for idx, i in enumerate(ks):
                                bank_id, region = av_region(a, i - i0)
                                nc.tensor.matmul(
                                    region,
                                    at[:, off + idx * 128: off + (idx + 1) * 128],
                                    lhs_v,
                                    start=not primed[bank_id],
                                    stop=(j == last_j[i]),
                                    skip_group_check=True,
                                )
                                primed[bank_id] = True

                    groups = []
                    sc_t = None
                    colw = 0
                    for j in range(NB):
                        ks = [i for i in col_kept[j] if i in irange]
                        if not ks:
                            continue
                        n = len(ks)
                        if sc_t is None or colw + n * 128 > FILL:
                            if sc_t is not None:
                                flush(groups, sc_t, colw)
                            sc_t = sc_ps.tile([128, FILL], F32, name="sc", tag="sc")
                            groups = []
                            colw = 0
                        for run in _runs_of(ks):
                            idx0 = ks.index(run[0])
                            col = colw + idx0 * 128
                            width = len(run) * 128
                            qcol = run[0] * 128
                            done = 0
                            while done < width:
                                seg = min(width - done, 512 - ((col + done) % 512))
                                nc.tensor.matmul(
                                    sc_t[:, col + done: col + done + seg],
                                    kT[p][rows, j * 128:(j + 1) * 128],
                                    qT[p][rows, qcol + done: qcol + done + seg],
                                )
                                done += seg
                        groups.append((j, ks, colw))
                        colw += n * 128
                    if sc_t is not None:
                        flush(groups, sc_t, colw)

                # ---- normalize + transpose this chunk (per head so head A's
                # transposes unlock while head B's normalize runs) ----
                rc = nrm_sb.tile([128, 8], F32, name="rc", tag="rc")
                nrm = nrm_sb.tile([128, 512], BF16, name="nrm", tag="nrm")
                if tp is None:
                    tp = alloc_tp()
                for a in range(2):
                    nc.vector.reciprocal(rc[:, 4 * a:4 * a + 4], av[a][:, 64:260:65])
                    nc.vector.tensor_tensor(
                        nrm[:, a * 256:a * 256 + 256].rearrange("p (r c) -> p r c", c=64),
                        av[a][:, 0:260].rearrange("p (r c) -> p r c", c=65)[:, :, 0:64],
                        rc[:, 4 * a:4 * a + 4].unsqueeze(2).broadcast_to([128, 4, 64]),
                        mybir.AluOpType.mult,
                    )
                    for li in range(QC):
                        idx = a * QC + li
                        nc.tensor.transpose(
                            tp[a * 64:(a + 1) * 64, li * 128:(li + 1) * 128],
                            nrm[:, idx * 64:(idx + 1) * 64],
                            ident[:],
                        )
                nc.vector.tensor_copy(
                    outTbf[p][:, qc * 512:(qc + 1) * 512], tp[:])

        def final(ms, tail=False):
            # mid-attention groups: copies on DVE only (Act must stay free for
            # exp); the post-attention tail group splits copies across engines
            # and borrows the idle av-pool banks for more psum parallelism
            for mi, m in enumerate(ms):
                fsb = fin_sb.tile([128, 1024], BF16, name="fsb", tag="fsb")
                for n in range(2):
                    if tail and (2 * mi + n) % 2 == 1:
                        ps = av_ps.tile([128, 512], F32, name="f_av",
                                        tag=f"av{(2 * mi + n) // 2 % 2}")
                    else:
                        ps = work_ps.tile([128, 512], F32, name="f_ps", tag="w")
                    for p in range(2):
                        nc.tensor.matmul(
                            ps[:],
                            outTbf[p][:, m * 128:(m + 1) * 128],
                            wo[:, p * 1024 + n * 512: p * 1024 + (n + 1) * 512],
                            start=(p == 0), stop=(p == 1),
                        )
                    if tail and n == 1:
                        nc.scalar.copy(fsb[:, n * 512:(n + 1) * 512], ps[:])
                    else:
                        nc.vector.tensor_copy(fsb[:, n * 512:(n + 1) * 512], ps[:])
                nc.sync.dma_start(
                    outp_ap[m * 128:(m + 1) * 128, :], fsb[:])

        # emission order = scheduler priority. Interleave p0/p1 chunks and emit
        # each final m-group right after the (p1,qc) that completes its outT
        # columns, so final matmuls act as PE filler while later chunks stall
        # on the activation engine (exp).
        # Emission order = scheduler priority: attention QK outranks the bulk
        # projections (v, qk p1) so the Act engine is fed scores ASAP; the
        # lower-priority projections + final groups then fill PE stalls.
        # emission order = scheduler priority AND psum slot FIFO order; see
        # ORDER spec tokens: qk0/qk1, vA (m0-7), vB (m8-15), aPQ, F0-F2, F3t
        for tok in _order():
            if tok.startswith("#"):
                continue
            if tok == "qk0":
                proj_qk(0)
            elif tok == "qk1":
                proj_qk(1)
            elif tok == "v":
                proj_v()
            elif tok == "vA":
                proj_v(range(0, 8))
            elif tok == "vB":
                proj_v(range(8, 16))
            elif tok.startswith("a"):
                attn_chunk(int(tok[1]), int(tok[2]))
            elif tok.startswith("F"):
                g = int(tok[1])
                final([4 * g + k for k in range(4)], tail=tok.endswith("t"))
            else:
                raise ValueError(tok)


def _get_nc(kept):
    key = (kept, ORDER)
    if key in _nc_cache:
        return _nc_cache[key]
    nc = bacc.Bacc("TRN2", target_bir_lowering=False, debug=False, num_devices=NCORES)
    xp_ap = nc.dram_tensor("xp", [128, 16384], BF16, kind="ExternalInput").ap()
    wq_ap = nc.dram_tensor("wqp", [128, 2048], BF16, kind="ExternalInput").ap()
    wk_ap = nc.dram_tensor("wkp", [128, 2048], BF16, kind="ExternalInput").ap()
    wv_ap = nc.dram_tensor("wvp", [128, 2048], BF16, kind="ExternalInput").ap()
    wo_ap = nc.dram_tensor("wop", [128, 2048], BF16, kind="ExternalInput").ap()
    id_ap = nc.dram_tensor("ident", [128, 128], BF16, kind="ExternalInput").ap()
    outp_ap = nc.dram_tensor("outp", [S, D], BF16, kind="ExternalOutput").ap()
    with tile.TileContext(nc) as tc:
        _emit(tc, (xp_ap, wq_ap, wk_ap, wv_ap, wo_ap, id_ap, outp_ap), kept)
    nc.compile()
    _nc_cache[key] = nc
    return nc


def _pack_x(xb):
    # x[b].T [1024,2048] -> [part, sc, kd, s-within] -> [128, 16384]
    t = np.ascontiguousarray(xb.T).astype(bf16)          # [1024, 2048]
    t = t.reshape(KD, 128, 4, 512).transpose(1, 2, 0, 3)  # [128, 4, 8, 512]
    return np.ascontiguousarray(t.reshape(128, 16384))


def _pack_w(wslT):
    # W[sl,:].T [1024, 256] -> [128, kd*256]
    t = wslT.reshape(KD, 128, 256).transpose(1, 0, 2)
    return np.ascontiguousarray(t.reshape(128, 2048)).astype(bf16)


def _pack_wo(woT):
    # Wo[:,sl].T [256, 1024] -> [128, p*1024 + outcol]
    t = woT.reshape(2, 128, 1024).transpose(1, 0, 2)
    return np.ascontiguousarray(t.reshape(128, 2048)).astype(bf16)


def kernel(x, Wq, Wk, Wv, Wo, bo, block_mask):
    x = np.asarray(x, dtype=np.float32)
    Wq = np.asarray(Wq, dtype=np.float32)
    Wk = np.asarray(Wk, dtype=np.float32)
    Wv = np.asarray(Wv, dtype=np.float32)
    Wo = np.asarray(Wo, dtype=np.float32)
    bo = np.asarray(bo, dtype=np.float32)
    mask = np.asarray(block_mask).astype(bool)

    kept = tuple(tuple(int(j) for j in np.nonzero(mask[i])[0]) for i in range(NB))
    assert all(len(js) > 0 for js in kept), "a query block row has no kept blocks"

    t0 = time.monotonic()
    nc = _get_nc(kept)
    t_compile = time.monotonic() - t0

    ident = np.eye(128).astype(bf16)
    xp_b = [_pack_x(x[b]) for b in range(B)]
    in_maps = []
    for c in range(NCORES):
        b = c // (NCORES // B)
        hs = c % (NCORES // B)
        sl = slice(hs * E, (hs + 1) * E)
        in_maps.append({
            "xp": xp_b[b],
            "wqp": _pack_w(np.ascontiguousarray(
                (Wq[sl, :] / np.sqrt(np.float32(DH))).T).astype(np.float32)),
            "wkp": _pack_w(np.ascontiguousarray(Wk[sl, :].T).astype(np.float32)),
            "wvp": _pack_w(np.ascontiguousarray(Wv[sl, :].T).astype(np.float32)),
            "wop": _pack_wo(np.ascontiguousarray(Wo[:, sl].T).astype(np.float32)),
            "ident": ident,
        })

    t0 = time.monotonic()
    res = run_bass_kernel_spmd(nc, in_maps, list(range(NCORES)))
    t_run = time.monotonic() - t0

    out = np.zeros((B, S, D), np.float32)
    for c in range(NCORES):
        out[c // (NCORES // B)] += res.results[c]["outp"].astype(np.float32)
    out += bo[None, None, :]

    last_run_info.update(compile_s=t_compile, run_s=t_run, nc=nc)
    return out


# revision 46
# speedup vs baseline: 1.7420x; 1.0070x over previous
"""Block-sparse attention on 8 Trainium2 NeuronCores (Bass/Tile SPMD kernel).

Sharding: batch*head_groups across the 8 cores. Core c handles batch c//4 and
heads [4*(c%4), 4*(c%4)+4). Projection weights are sliced per core host-side
(pre-transposed + bf16-cast, packed so each weight is ONE dma); the [16,16]
block mask specializes the compiled program (only kept blocks are computed).
Each core emits a partial output (its 256-wide d-slice pushed through Wo) in
bf16; the host sums the 4 partials per batch in f32 and adds the bias.

Design (driven by the TimelineSim cost model, where matmul cost = N_out only):
  - scores [k, q] via stationary k^T block (K=64), moving q^T runs (N=128/blk)
  - exp on Act engine into bf16 `at` tiles (the overall pacer: ~0.83ns/elem)
  - AV *natural*: stationary = at block [128k,128q], moving = [v_h|1] [128,65]
    -> av psum [128 q, 65] accumulated over kept j (N=65/blk, den for free)
  - batched normalize: 1/den per-region (DVE reciprocal on strided AP), one
    broadcast (0-stride) tensor_tensor multiply per av bank -> norm_sb bf16
  - PE transpose (bf16 identity) norm_sb [128q,64d] -> tp psum bf16 [64,128],
    packed 2 heads x 4 blocks per [128,512]; one DVE 2x copy -> outT
  - final: outT stationary, wo moving, f32 psum -> bf16 sb -> 1 dma per m
PSUM: sc pool 2x[128,1024]f32 (4 banks, also hosts tp tiles), av 2 banks
(65-wide regions packed 7/bank per 4-i-row chunk), work pool 2 banks
(v/qk-p1 projections + final) = 8. The tile list-scheduler overlaps
projection/final PE work under the Act-bound attention automatically.
"""

import time
from contextlib import ExitStack

import ml_dtypes
import numpy as np

import concourse.bass as bass
import concourse.tile as tile
from concourse import bacc, mybir
from concourse.bass_utils import run_bass_kernel_spmd

BF16 = mybir.dt.bfloat16
F32 = mybir.dt.float32
bf16 = ml_dtypes.bfloat16

B, S, D, H = 2, 2048, 1024, 16
DH = 64
BLK = 128
NB = 16
NCORES = 8
HPC = H // (NCORES // B)   # 4 heads per core
E = HPC * DH               # 256 projection columns per core
EV = HPC * (DH + 1)        # 260: v stored as [v_h | 1] per head
KD = D // 128              # 8 contraction chunks
QC = 4                     # query-block rows per attention chunk
NQC = NB // QC             # 4 chunks
FILL = 1024                # score fill width (2 psum banks, 8 block-cols)

_nc_cache: dict = {}
last_run_info: dict = {}
AT_BUFS = 28
TP_IN_WORK = True
V_PRIO = 0
TP_AT_END = False

# NOTE: emission order IS program order for the tile dep tracker — every
# tensor's writer must be emitted before its readers (v before all attention,
# qk1 before a1*, Fg after both g-chunks). high_priority on attention makes
# the low-priority proj/final work execute lazily as PE gap filler anyway.
ORDER = ("qk0", "v", "a00", "a01", "a02", "a03", "qk1", "a10",
         "F0", "a11", "F1", "a12", "F2", "a13", "F3t")


def _order():
    return ORDER


def _runs_of(lst):
    out = []
    for i in lst:
        if out and i == out[-1][-1] + 1:
            out[-1].append(i)
        else:
            out.append([i])
    return out


def _emit(tc, aps, kept):
    nc = tc.nc
    xp_ap, wq_ap, wk_ap, wv_ap, wo_ap, id_ap, outp_ap = aps
    Exp = mybir.ActivationFunctionType.Exp
    first_j = {i: kept[i][0] for i in range(NB)}
    last_j = {i: kept[i][-1] for i in range(NB)}
    col_kept = [[i for i in range(NB) if j in kept[i]] for j in range(NB)]

    with ExitStack() as ctx:
        persist = ctx.enter_context(tc.tile_pool(name="persist", bufs=1))
        sc_ps = ctx.enter_context(tc.tile_pool(name="sc_ps", bufs=2, space="PSUM"))
        work_ps = ctx.enter_context(tc.tile_pool(name="work_ps", bufs=2, space="PSUM"))
        av_ps = ctx.enter_context(tc.tile_pool(name="av_ps", bufs=1, space="PSUM"))
        at_sb = ctx.enter_context(tc.tile_pool(name="at_sb", bufs=AT_BUFS))
        nrm_sb = ctx.enter_context(tc.tile_pool(name="nrm_sb", bufs=4))
        fin_sb = ctx.enter_context(tc.tile_pool(name="fin_sb", bufs=4))

        # ---- input loads: wq then x0 first so q-proj sc0 starts earliest ----
        ident = persist.tile([128, 128], BF16, name="ident", tag="ident")
        wq = persist.tile([128, 2048], BF16, name="wq", tag="wq")
        wk = persist.tile([128, 2048], BF16, name="wk", tag="wk")
        wv = persist.tile([128, 2048], BF16, name="wv", tag="wv")
        wo = persist.tile([128, 2048], BF16, name="wo", tag="wo")
        # x packed [part, sc, kd, s]: one tile per s-group so deps are exact
        xg = [persist.tile([128, 4096], BF16, name=f"xg{g}", tag=f"xg{g}")
              for g in range(4)]
        nc.sync.dma_start(wq[:], wq_ap[:, :])
        nc.sync.dma_start(xg[0][:, 0:2048], xp_ap[:, 0:2048])
        nc.sync.dma_start(xg[0][:, 2048:4096], xp_ap[:, 2048:4096])
        nc.sync.dma_start(wk[:], wk_ap[:, :])
        nc.sync.dma_start(xg[1][:], xp_ap[:, 4096:8192])
        nc.sync.dma_start(wv[:], wv_ap[:, :])
        nc.sync.dma_start(xg[2][:], xp_ap[:, 8192:12288])
        nc.sync.dma_start(wo[:], wo_ap[:, :])
        nc.sync.dma_start(ident[:], id_ap[:, :])
        nc.sync.dma_start(xg[3][:], xp_ap[:, 12288:16384])

        qT = [persist.tile([128, S], BF16, name=f"qT{p}", tag=f"qT{p}") for p in range(2)]
        kT = [persist.tile([128, S], BF16, name=f"kT{p}", tag=f"kT{p}") for p in range(2)]
        vv = [persist.tile([128, EV], BF16, name=f"v{m}", tag=f"v{m}") for m in range(NB)]
        outTbf = [persist.tile([128, S], BF16, name=f"oT{p}", tag=f"oT{p}") for p in range(2)]

        def x_mv(kd, sc):
            # moving x^T slice for contraction chunk kd, s-columns [512*sc, 512*(sc+1))
            return xg[sc][:, kd * 512:(kd + 1) * 512]

        def proj_qk(p):
            # sc outer so each x-group is fully consumed as it arrives off DMA
            for sc in range(4):
                for w, dst in ((wq, qT[p]), (wk, kT[p])):
                    ps = work_ps.tile([128, 512], F32, name="w_ps", tag="w")
                    for kd in range(KD):
                        nc.tensor.matmul(
                            ps[:],
                            w[:, kd * 256 + p * 128: kd * 256 + (p + 1) * 128],
                            x_mv(kd, sc),
                            start=(kd == 0), stop=(kd == KD - 1),
                        )
                    nc.vector.tensor_copy(dst[:, sc * 512:(sc + 1) * 512], ps[:])

        def proj_v(ms=range(NB)):
            from contextlib import nullcontext
            pctx = tc.high_priority(offset=V_PRIO) if V_PRIO else nullcontext()
            with pctx:
                _proj_v_body(ms)

        def _proj_v_body(ms):
            for m in ms:
                ps = work_ps.tile([128, 260], F32, name="v_ps", tag="w")
                for kd in range(KD):
                    nc.tensor.matmul(
                        ps[:, 0:E],
                        xg[m // 4][:, kd * 512 + (m % 4) * 128:
                                   kd * 512 + (m % 4) * 128 + 128],
                        wv[:, kd * 256:(kd + 1) * 256],
                        start=(kd == 0), stop=(kd == KD - 1),
                    )
                v3 = vv[m].rearrange("p (g c) -> p g c", g=HPC)
                nc.any.memset(v3[:, :, 64:65], 1.0)
                nc.vector.tensor_copy(
                    v3[:, :, 0:64],
                    ps[:, 0:E].rearrange("p (g c) -> p g c", g=HPC),
                )

        def attn_chunk(p, qc):
            # attention always outranks proj/final filler in the scheduler's
            # ready heap (dependencies still force projections to run first
            # where needed); emission order still controls psum slot FIFO.
            with tc.high_priority(offset=1_000_000):
                i0 = qc * QC
                irange = list(range(i0, i0 + QC))
                # av accumulators: one bank per head, 4 regions of width 65
                av = [av_ps.tile([128, 512], F32, name=f"av{a}", tag=f"av{a}")
                      for a in range(2)]

                # start=True pending-zeroes the whole 2KB bank (zero region),
                # so only the FIRST matmul touching each bank may use it; all
                # other regions' first writes then read-as-zero via the mark.
                primed = [False, False]

                def av_region(a, li):
                    return a, av[a][:, 65 * li:65 * li + 65]

                # plan flushes: exact 8-block packing (j-groups may split) so
                # exp count is minimal; then emit head A/B flushes interleaved
                # so head A's normalize overlaps head B's remaining flushes
                blocks = [(j, i) for j in range(NB)
                          for i in col_kept[j] if i in irange]
                CAP = FILL // 128
                plans = [blocks[c:c + CAP] for c in range(0, len(blocks), CAP)]

                def emit_qk(a, fl):
                    rows = slice(a * 64, (a + 1) * 64)
                    sc_t = sc_ps.tile([128, FILL], F32, name="sc", tag="sc")
                    # fuse consecutive same-j runs of consecutive i into one
                    # matmul, split at 512-col psum bank boundaries
                    col = 0
                    for j, ks in fl:
                        for run in _runs_of(ks):
                            width = len(run) * 128
                            qcol = run[0] * 128
                            done = 0
                            while done < width:
                                seg = min(width - done, 512 - ((col + done) % 512))
                                nc.tensor.matmul(
                                    sc_t[:, col + done: col + done + seg],
                                    kT[p][rows, j * 128:(j + 1) * 128],
                                    qT[p][rows, qcol + done: qcol + done + seg],
                                )
                                done += seg
                            col += width
                    return sc_t, col

                def emit_exp_av(a, fl, sc_t, colw):
                    h = 2 * p + a
                    at = at_sb.tile([128, FILL], BF16, name="at", tag="at")
                    nc.scalar.activation(at[:, 0:colw], sc_t[:, 0:colw], Exp)
                    off = 0
                    for j, ks in fl:
                        lhs_v = vv[j][:, 65 * h:65 * h + 65]
                        for i in ks:
                            bank_id, region = av_region(a, i - i0)
                            nc.tensor.matmul(
                                region,
                                at[:, off:off + 128],
                                lhs_v,
                                start=not primed[bank_id],
                                stop=(j == last_j[i]),
                                skip_group_check=True,
                            )
                            primed[bank_id] = True
                            off += 128

                for fi, plan in enumerate(plans):
                    # regroup plan entries [(j,i)...] -> [(j, ks)...]
                    fl = []
                    for j, i in plan:
                        if fl and fl[-1][0] == j:
                            fl[-1][1].append(i)
                        else:
                            fl.append((j, [i]))
                    for a in range(2):
                        sc_t, colw = emit_qk(a, fl)
                        emit_exp_av(a, fl, sc_t, colw)

                # ---- normalize + transpose this chunk (per head so head A's
                # transposes unlock while head B's normalize runs) ----
                rc = nrm_sb.tile([128, 8], F32, name="rc", tag="rc")
                nrm = nrm_sb.tile([128, 512], BF16, name="nrm", tag="nrm")
                tp = work_ps.tile([128, 512], BF16, name="tp", tag="w")
                for a in range(2):
                    nc.vector.reciprocal(rc[:, 4 * a:4 * a + 4], av[a][:, 64:260:65])
                    nc.vector.tensor_tensor(
                        nrm[:, a * 256:a * 256 + 256].rearrange("p (r c) -> p r c", c=64),
                        av[a][:, 0:260].rearrange("p (r c) -> p r c", c=65)[:, :, 0:64],
                        rc[:, 4 * a:4 * a + 4].unsqueeze(2).broadcast_to([128, 4, 64]),
                        mybir.AluOpType.mult,
                    )
                    for li in range(QC):
                        idx = a * QC + li
                        nc.tensor.transpose(
                            tp[a * 64:(a + 1) * 64, li * 128:(li + 1) * 128],
                            nrm[:, idx * 64:(idx + 1) * 64],
                            ident[:],
                        )
                nc.vector.tensor_copy(
                    outTbf[p][:, qc * 512:(qc + 1) * 512], tp[:])

        def final(ms, tail=False):
            # mid-attention groups: copies on DVE only (Act must stay free for
            # exp); the post-attention tail group splits copies across engines
            # and borrows the idle av-pool banks for more psum parallelism
            for mi, m in enumerate(ms):
                fsb = fin_sb.tile([128, 1024], BF16, name="fsb", tag="fsb")
                for n in range(2):
                    if tail and (2 * mi + n) % 2 == 1:
                        ps = av_ps.tile([128, 512], F32, name="f_av",
                                        tag=f"av{(2 * mi + n) // 2 % 2}")
                    else:
                        ps = work_ps.tile([128, 512], F32, name="f_ps", tag="w")
                    for p in range(2):
                        nc.tensor.matmul(
                            ps[:],
                            outTbf[p][:, m * 128:(m + 1) * 128],
                            wo[:, p * 1024 + n * 512: p * 1024 + (n + 1) * 512],
                            start=(p == 0), stop=(p == 1),
                        )
                    if tail and n == 1:
                        nc.scalar.copy(fsb[:, n * 512:(n + 1) * 512], ps[:])
                    else:
                        nc.vector.tensor_copy(fsb[:, n * 512:(n + 1) * 512], ps[:])
                nc.sync.dma_start(
                    outp_ap[m * 128:(m + 1) * 128, :], fsb[:])

        # emission order = scheduler priority. Interleave p0/p1 chunks and emit
        # each final m-group right after the (p1,qc) that completes its outT
        # columns, so final matmuls act as PE filler while later chunks stall
        # on the activation engine (exp).
        # Emission order = scheduler priority: attention QK outranks the bulk
        # projections (v, qk p1) so the Act engine is fed scores ASAP; the
        # lower-priority projections + final groups then fill PE stalls.
        # emission order = scheduler priority AND psum slot FIFO order; see
        # ORDER spec tokens: qk0/qk1, vA (m0-7), vB (m8-15), aPQ, F0-F2, F3t
        for tok in _order():
            if tok.startswith("#"):
                continue
            if tok == "qk0":
                proj_qk(0)
            elif tok == "qk1":
                proj_qk(1)
            elif tok == "v":
                proj_v()
            elif tok == "vA":
                proj_v(range(0, 8))
            elif tok == "vB":
                proj_v(range(8, 16))
            elif tok.startswith("a"):
                attn_chunk(int(tok[1]), int(tok[2]))
            elif tok.startswith("F"):
                g = int(tok[1])
                final([4 * g + k for k in range(4)], tail=tok.endswith("t"))
            else:
                raise ValueError(tok)


def _get_nc(kept):
    key = (kept, ORDER)
    if key in _nc_cache:
        return _nc_cache[key]
    nc = bacc.Bacc("TRN2", target_bir_lowering=False, debug=False, num_devices=NCORES)
    xp_ap = nc.dram_tensor("xp", [128, 16384], BF16, kind="ExternalInput").ap()
    wq_ap = nc.dram_tensor("wqp", [128, 2048], BF16, kind="ExternalInput").ap()
    wk_ap = nc.dram_tensor("wkp", [128, 2048], BF16, kind="ExternalInput").ap()
    wv_ap = nc.dram_tensor("wvp", [128, 2048], BF16, kind="ExternalInput").ap()
    wo_ap = nc.dram_tensor("wop", [128, 2048], BF16, kind="ExternalInput").ap()
    id_ap = nc.dram_tensor("ident", [128, 128], BF16, kind="ExternalInput").ap()
    outp_ap = nc.dram_tensor("outp", [S, D], BF16, kind="ExternalOutput").ap()
    with tile.TileContext(nc) as tc:
        _emit(tc, (xp_ap, wq_ap, wk_ap, wv_ap, wo_ap, id_ap, outp_ap), kept)
    nc.compile()
    _nc_cache[key] = nc
    return nc


def _pack_x(xb):
    # x[b].T [1024,2048] -> [part, sc, kd, s-within] -> [128, 16384]
    t = np.ascontiguousarray(xb.T).astype(bf16)          # [1024, 2048]
    t = t.reshape(KD, 128, 4, 512).transpose(1, 2, 0, 3)  # [128, 4, 8, 512]
    return np.ascontiguousarray(t.reshape(128, 16384))


def _pack_w(wslT):
    # W[sl,:].T [1024, 256] -> [128, kd*256]
    t = wslT.reshape(KD, 128, 256).transpose(1, 0, 2)
    return np.ascontiguousarray(t.reshape(128, 2048)).astype(bf16)


def _pack_wo(woT):
    # Wo[:,sl].T [256, 1024] -> [128, p*1024 + outcol]
    t = woT.reshape(2, 128, 1024).transpose(1, 0, 2)
    return np.ascontiguousarray(t.reshape(128, 2048)).astype(bf16)


def kernel(x, Wq, Wk, Wv, Wo, bo, block_mask):
    x = np.asarray(x, dtype=np.float32)
    Wq = np.asarray(Wq, dtype=np.float32)
    Wk = np.asarray(Wk, dtype=np.float32)
    Wv = np.asarray(Wv, dtype=np.float32)
    Wo = np.asarray(Wo, dtype=np.float32)
    bo = np.asarray(bo, dtype=np.float32)
    mask = np.asarray(block_mask).astype(bool)

    kept = tuple(tuple(int(j) for j in np.nonzero(mask[i])[0]) for i in range(NB))
    assert all(len(js) > 0 for js in kept), "a query block row has no kept blocks"

    t0 = time.monotonic()
    nc = _get_nc(kept)
    t_compile = time.monotonic() - t0

    ident = np.eye(128).astype(bf16)
    xp_b = [_pack_x(x[b]) for b in range(B)]
    in_maps = []
    for c in range(NCORES):
        b = c // (NCORES // B)
        hs = c % (NCORES // B)
        sl = slice(hs * E, (hs + 1) * E)
        in_maps.append({
            "xp": xp_b[b],
            "wqp": _pack_w(np.ascontiguousarray(
                (Wq[sl, :] / np.sqrt(np.float32(DH))).T).astype(np.float32)),
            "wkp": _pack_w(np.ascontiguousarray(Wk[sl, :].T).astype(np.float32)),
            "wvp": _pack_w(np.ascontiguousarray(Wv[sl, :].T).astype(np.float32)),
            "wop": _pack_wo(np.ascontiguousarray(Wo[:, sl].T).astype(np.float32)),
            "ident": ident,
        })

    t0 = time.monotonic()
    res = run_bass_kernel_spmd(nc, in_maps, list(range(NCORES)))
    t_run = time.monotonic() - t0

    out = np.zeros((B, S, D), np.float32)
    for c in range(NCORES):
        out[c // (NCORES // B)] += res.results[c]["outp"].astype(np.float32)
    out += bo[None, None, :]

    last_run_info.update(compile_s=t_compile, run_s=t_run, nc=nc)
    return out
